# revision 11
# baseline (speedup 1.0000x reference)
"""BigBird block-sliding-window attention on 8 TRN2 NeuronCores.

Sharding: batch (2) x sequence-quarters (4) -> 8 shards of 1024 tokens.
Each core gets its 8 query blocks plus a 1-block halo of x on each side
(zero-padded at sequence ends), computes Q/K/V projections, block-local
attention over the 3-block windows, and the output projection for its
token range.  No collectives needed: padded-zero keys contribute
exp(0)=1 to the softmax denominator exactly as in the reference
(projections are bias-free, so zero x rows give zero k/v rows).

Device layout notes:
  - x is passed pre-transposed (xT [DIM, 1280] bf16) so the contraction
    dim (DIM) is on partitions for all projection matmuls.
  - q/k are produced transposed (qT/kT [64*2heads, tok]) by using Wq/Wk
    chunks as the stationary operand; v and the final output are
    produced in natural [token, feature] layout.
  - scores S = qT.T @ kT_window ([128 qtok, 384]); softmax via one ACT
    Exp with accum_out row sums; P normalized then transposed on the PE
    (3x 128x128) so the AV matmuls can contract over key tokens,
    producing A^T = attention-output-transposed, which feeds the output
    projection directly as the stationary operand.
"""

import numpy as np
import ml_dtypes

BF16 = ml_dtypes.bfloat16

DIM, H, DK, DV, BS = 1536, 8, 64, 192, 128
B, N = 2, 4096
NB = 10            # blocks per core including 1-block halo each side
NT = NB * BS       # 1280 tokens of x per core
NQ = 8             # central query blocks per core
ND = DIM // 128    # 12 chunks of the contraction dim
NCORES = 8

_CACHE = {}


def _build_bass():
    import concourse.bass as bass
    import concourse.mybir as mybir
    import concourse.tile as tile
    from concourse.vector_clock import ScopedClock
    from concourse.masks import make_identity

    dt = mybir.dt
    AF = mybir.ActivationFunctionType

    class PatchedTileContext(tile.TileContext):
        # The walrus in this image rejects instructions carrying more than
        # one sync wait; hoist extras onto single-wait NOPs placed just
        # before the instruction on the same engine (engine executes in
        # program order, so wait-then-wait == wait-on-both).
        def _add_instruction(self, inst):
            si = inst.sync_info
            if (
                si is not None
                and si.on_wait
                and len(si.on_wait) > 1
                and inst.engine != mybir.EngineType.Unassigned
            ):
                waits = list(si.on_wait)
                si.on_wait = [waits[-1]]
                for w in waits[:-1]:
                    nop = mybir.InstNoOp(
                        name=f"I-{self.nc.next_id()}",
                        ins=[],
                        outs=[],
                        bass_nofuse=True,
                    )
                    nop.engine = inst.engine
                    nop.sync_info = mybir.SyncInfo(on_wait=[w], on_update=[])
                    super()._add_instruction(nop)
            super()._add_instruction(inst)

        def _drain_and_barrier(self, tick_clock, wait_clock):
            carrier = self.nc.sync.nop(nofuse=True)
            wait_clock.add_sem_waits(
                carrier.ins, ScopedClock({None: tick_clock.global_clock})
            )
            si = carrier.ins.sync_info
            waits = list(si.on_wait) if si and si.on_wait else []
            if len(waits) > 1:
                si.on_wait = [waits[0]]
                for w in waits[1:]:
                    extra = self.nc.sync.nop(nofuse=True)
                    esi = extra.ins.sync_info
                    if esi is None:
                        extra.ins.sync_info = mybir.SyncInfo(
                            on_wait=[w], on_update=[]
                        )
                    else:
                        esi.on_wait = [w]
            self.nc.sync.drain()
            self.nc.all_engine_barrier()
            popped = self.nc._tile_sem_poison_stack.pop()
            assert popped is self._sem_poison
            self.nc.clear_and_free_semaphores(
                list(self.sems.allocated().values())
            )
            self.nc.all_engine_barrier()

    nc = bass.Bass(
        "TRN2", target_bir_lowering=False, debug=False, num_devices=NCORES
    )
    xT_d = nc.dram_tensor("xT", [DIM, NT], dt.bfloat16, kind="ExternalInput")
    Wq_d = nc.dram_tensor("Wq", [DIM, H * DK], dt.bfloat16, kind="ExternalInput")
    Wk_d = nc.dram_tensor("Wk", [DIM, H * DK], dt.bfloat16, kind="ExternalInput")
    Wv_d = nc.dram_tensor("Wv", [DIM, H * DV], dt.bfloat16, kind="ExternalInput")
    Wo_d = nc.dram_tensor("Wo", [H * DV, DIM], dt.bfloat16, kind="ExternalInput")
    bo_d = nc.dram_tensor("bo_b", [128, DIM], dt.float32, kind="ExternalInput")
    y_d = nc.dram_tensor("y", [NQ * BS, DIM], dt.float32, kind="ExternalOutput")

    with PatchedTileContext(nc) as tc:
        with (
            tc.tile_pool(name="res", bufs=1) as res,
            tc.tile_pool(name="vpool", bufs=5) as vpool,
            tc.tile_pool(name="work", bufs=3) as work,
            tc.tile_pool(name="small", bufs=4) as small,
            tc.tile_pool(name="outp", bufs=2) as outp,
            tc.tile_pool(name="pp", bufs=2, space="PSUM") as pp,
            tc.tile_pool(name="ps", bufs=2, space="PSUM") as ps,
            tc.tile_pool(name="ppt", bufs=2, space="PSUM") as ppt,
            tc.tile_pool(name="pat", bufs=1, space="PSUM") as pat,
        ):
            # ---- resident tiles + input DMA ----
            xT_sb = res.tile([128, ND, NT], dt.bfloat16, tag="xT")
            Wq_sb = res.tile([128, ND, H * DK], dt.bfloat16, tag="Wq")
            Wk_sb = res.tile([128, ND, H * DK], dt.bfloat16, tag="Wk")
            Wv_sb = res.tile([128, ND, H * DV], dt.bfloat16, tag="Wv")
            Wo_sb = res.tile([128, ND, DIM], dt.bfloat16, tag="Wo")
            bo_sb = res.tile([128, DIM], dt.float32, tag="bo")
            kT_sb = res.tile([128, H // 2, NT], dt.bfloat16, tag="kT")
            qT_sb = res.tile([128, H // 2, NQ * BS], dt.bfloat16, tag="qT")
            ident = res.tile([128, 128], dt.bfloat16, tag="ident")

            make_identity(nc, ident[:])
            # DMA order is the startup critical path: the k/q projections
            # need xT (first token group) + Wk/Wq first.  All transfers go
            # through the SWDGE queue (the HWDGE queue on this image runs
            # at ~2.7 GB/s per engine); spread the enqueue work across
            # three engines so the critical loads are in flight early.
            for d in range(ND):
                nc.scalar.dma_start(
                    xT_sb[:, d, 0:512], xT_d[d * 128:(d + 1) * 128, 0:512]
                )
            for d in range(ND):
                nc.scalar.dma_start(Wk_sb[:, d, :], Wk_d[d * 128:(d + 1) * 128, :])
            for d in range(ND):
                nc.gpsimd.dma_start(Wq_sb[:, d, :], Wq_d[d * 128:(d + 1) * 128, :])
            for tok0, w in [(512, 512), (1024, 256)]:
                for d in range(ND):
                    nc.scalar.dma_start(
                        xT_sb[:, d, tok0:tok0 + w],
                        xT_d[d * 128:(d + 1) * 128, tok0:tok0 + w],
                    )
            for d in range(ND):
                nc.gpsimd.dma_start(Wv_sb[:, d, :], Wv_d[d * 128:(d + 1) * 128, :])
            nc.gpsimd.dma_start(bo_sb[:], bo_d[:])
            for d in range(ND):
                nc.gpsimd.dma_start(Wo_sb[:, d, :], Wo_d[d * 128:(d + 1) * 128, :])

            # ---- transposed projections (kT over all 10 blocks, qT over
            #      the 8 central blocks); head pair hp holds heads
            #      (2hp, 2hp+1) at partitions (0:64, 64:128) ----
            def proj_T(dst_sb, W_sb, groups, toff0):
                for tok0, w in groups:
                    for hp in range(H // 2):
                        acc = pp.tile([128, 512], dt.float32, tag="pp")
                        for d in range(ND):
                            nc.tensor.matmul(
                                acc[:, :w],
                                W_sb[:, d, hp * 128:(hp + 1) * 128],
                                xT_sb[:, d, tok0:tok0 + w],
                                start=(d == 0),
                                stop=(d == ND - 1),
                            )
                        nc.scalar.copy(
                            dst_sb[:, hp, tok0 - toff0:tok0 - toff0 + w],
                            acc[:, :w],
                        )

            proj_T(kT_sb, Wk_sb, [(0, 512), (512, 512), (1024, 256)], 0)
            proj_T(qT_sb, Wq_sb, [(128, 512), (640, 512)], 128)

            # ---- v projection for one block (natural [tok, h*dv]) ----
            def v_proj(blk):
                vt = vpool.tile([128, H * DV], dt.bfloat16, tag="v")
                for vc in range(3):
                    acc = pp.tile([128, 512], dt.float32, tag="pp")
                    for d in range(ND):
                        nc.tensor.matmul(
                            acc[:],
                            xT_sb[:, d, blk * 128:(blk + 1) * 128],
                            Wv_sb[:, d, vc * 512:(vc + 1) * 512],
                            start=(d == 0),
                            stop=(d == ND - 1),
                        )
                    nc.vector.tensor_copy(vt[:, vc * 512:(vc + 1) * 512], acc[:])
                return vt

            # ---- attention + output projection for query blocks
            #      (qb, qb+1); AV matmuls are batched over the two query
            #      blocks (the shared k-blocks contribute to both via a
            #      strided rhs AP, N=256) ----
            def attention_pair(qb, vts):
                at_sbs = {
                    q2: work.tile(
                        [128, H * DV], dt.bfloat16, tag="at", name="at_sb"
                    )
                    for q2 in (qb, qb + 1)
                }
                for g in range(H // 2):
                    h0c, h1c = 2 * g * DV, (2 * g + 1) * DV
                    pts = []
                    for sub in range(2):
                        p0 = 64 * sub
                        # pt_ps columns: (qb: c0 c1 c2)(qb+1: c0 c1 c2)
                        pt_ps = ppt.tile([128, 768], dt.bfloat16, tag="ppt")
                        for qbi in range(2):
                            q2 = qb + qbi
                            s_ps = ps.tile([128, 384], dt.float32, tag="ps")
                            nc.tensor.matmul(
                                s_ps[:],
                                qT_sb[p0:p0 + 64, g, (q2 - 1) * 128:q2 * 128],
                                kT_sb[p0:p0 + 64, g,
                                      (q2 - 1) * 128:(q2 + 2) * 128],
                                start=True,
                                stop=True,
                            )
                            p_sb = work.tile([128, 384], dt.float32, tag="p")
                            ssum = small.tile([128, 1], dt.float32, tag="ssum")
                            nc.scalar.activation(
                                p_sb[:], s_ps[:], AF.Exp,
                                bias=0.0, scale=0.125, accum_out=ssum[:],
                            )
                            rinv = small.tile([128, 1], dt.float32, tag="rinv")
                            nc.vector.reciprocal(rinv[:], ssum[:])
                            pn_sb = work.tile([128, 384], dt.bfloat16, tag="pn")
                            nc.vector.tensor_scalar_mul(
                                pn_sb[:], p_sb[:], rinv[:]
                            )
                            for c in range(3):
                                nc.tensor.transpose(
                                    pt_ps[:, qbi * 384 + c * 128:
                                          qbi * 384 + (c + 1) * 128],
                                    pn_sb[:, c * 128:(c + 1) * 128],
                                    ident[:],
                                )
                        pt_sb = work.tile([128, 768], dt.bfloat16, tag="pt")
                        nc.scalar.copy(pt_sb[:], pt_ps[:])
                        pts.append(pt_sb)

                    # A^T psum for this pair: chunk r at cols r*256, holding
                    # (qb | qb+1) side by side; chunk 0 = h0 dv0:128,
                    # chunk 1 = h0 dv128:192 (parts 0:64) + h1 dv0:64
                    # (parts 64:128), chunk 2 = h1 dv64:192.
                    at_ps = pat.tile([128, 768], dt.float32, tag="pat")
                    chains = [
                        (0, h0c, 128, 0, 0, 128, None),
                        (0, h0c + 128, 64, 256, 0, 64, None),
                        (1, h1c, 64, 256, 64, 128, (0, 64)),
                        (1, h1c + 64, 128, 512, 0, 128, None),
                    ]
                    for sub, vc0, wdt, oc0, plo, phi, tpos in chains:
                        pt_sb = pts[sub]
                        pt_re = pt_sb[:].rearrange(
                            "p (a b c) -> p b a c", a=3, b=2, c=128
                        )
                        for j in range(4):
                            vt = vts[qb - 1 + j]
                            if j == 0:
                                rhs = pt_sb[:, 0:128]
                                out_ap = at_ps[plo:phi, oc0:oc0 + 128]
                            elif j == 1:
                                rhs = pt_re[:, 1, 0:2, :]
                                out_ap = at_ps[plo:phi, oc0:oc0 + 256]
                            elif j == 2:
                                rhs = pt_re[:, 0, 1:3, :]
                                out_ap = at_ps[plo:phi, oc0:oc0 + 256]
                            else:
                                rhs = pt_sb[:, 640:768]
                                out_ap = at_ps[plo:phi, oc0 + 128:oc0 + 256]
                            nc.tensor.matmul(
                                out_ap,
                                vt[:, vc0:vc0 + wdt],
                                rhs,
                                start=(j == 0),
                                stop=(j == 3),
                                tile_position=tpos,
                            )
                    at_re = at_ps[:].rearrange(
                        "p (r q x) -> p q r x", r=3, q=2, x=128
                    )
                    for qbi in range(2):
                        nc.vector.tensor_copy(
                            at_sbs[qb + qbi][:, g * 384:(g + 1) * 384],
                            at_re[:, qbi],
                        )

                for q2 in (qb, qb + 1):
                    at_sb = at_sbs[q2]
                    o_sb = outp.tile([128, DIM], dt.float32, tag="o")
                    for n in range(3):
                        acc = pp.tile([128, 512], dt.float32, tag="pp")
                        for j in range(ND):
                            nc.tensor.matmul(
                                acc[:],
                                at_sb[:, j * 128:(j + 1) * 128],
                                Wo_sb[:, j, n * 512:(n + 1) * 512],
                                start=(j == 0),
                                stop=(j == ND - 1),
                            )
                        nc.vector.tensor_add(
                            o_sb[:, n * 512:(n + 1) * 512],
                            acc[:],
                            bo_sb[:, n * 512:(n + 1) * 512],
                        )
                    nc.gpsimd.dma_start(y_d[(q2 - 1) * 128:q2 * 128, :], o_sb[:])

            vts = {}
            for blk in range(3):
                vts[blk] = v_proj(blk)
            vts[3] = v_proj(3)
            attention_pair(1, vts)
            for bp in range(1, 4):
                qb = 1 + 2 * bp
                vts[qb + 1] = v_proj(qb + 1)
                vts[qb + 2] = v_proj(qb + 2)
                attention_pair(qb, vts)

    return nc


def _get_nc():
    if "nc" not in _CACHE:
        _CACHE["nc"] = _build_bass()
    return _CACHE["nc"]


def _shard_inputs(x, Wq, Wk, Wv, Wo, bo):
    xp = np.zeros((B, N + 2 * BS, DIM), np.float32)
    xp[:, BS:BS + N] = np.asarray(x, np.float32)
    Wq_b = np.asarray(Wq).astype(BF16)
    Wk_b = np.asarray(Wk).astype(BF16)
    Wv_b = np.asarray(Wv).astype(BF16)
    Wo_b = np.asarray(Wo).astype(BF16)
    bo_b = np.ascontiguousarray(
        np.broadcast_to(np.asarray(bo, np.float32), (128, DIM))
    )
    in_maps = []
    for c in range(NCORES):
        b, ch = divmod(c, 4)
        xs = xp[b, ch * NQ * BS: ch * NQ * BS + NT]  # [1280, 1536]
        xT = np.ascontiguousarray(xs.T).astype(BF16)
        in_maps.append({
            "xT": xT, "Wq": Wq_b, "Wk": Wk_b, "Wv": Wv_b, "Wo": Wo_b,
            "bo_b": bo_b,
        })
    return in_maps


def kernel(x, Wq, Wk, Wv, Wo, bo):
    from concourse.bass_utils import run_bass_kernel_spmd

    nc = _get_nc()
    in_maps = _shard_inputs(x, Wq, Wk, Wv, Wo, bo)
    res = run_bass_kernel_spmd(nc, in_maps, list(range(NCORES)))
    out = np.empty((B, N, DIM), np.float32)
    for c in range(NCORES):
        b, ch = divmod(c, 4)
        out[b, ch * NQ * BS:(ch + 1) * NQ * BS] = res.results[c]["y"]
    return out


# revision 12
# speedup vs baseline: 1.1452x; 1.1452x over previous
"""BigBird block-sliding-window attention on 8 TRN2 NeuronCores.

Sharding: batch (2) x sequence-quarters (4) -> 8 shards of 1024 tokens.
Each core gets its 8 query blocks plus a 1-block halo of x on each side
(zero-padded at sequence ends), computes Q/K/V projections, block-local
attention over the 3-block windows, and the output projection for its
token range.  No collectives needed: padded-zero keys contribute
exp(0)=1 to the softmax denominator exactly as in the reference
(projections are bias-free, so zero x rows give zero k/v rows).

Device layout notes:
  - x is passed pre-transposed (xT [DIM, 1280] bf16) so the contraction
    dim (DIM) is on partitions for all projection matmuls.
  - q/k are produced transposed (qT/kT [64*2heads, tok]) by using Wq/Wk
    chunks as the stationary operand; v and the final output are
    produced in natural [token, feature] layout.
  - scores S = qT.T @ kT_window ([128 qtok, 384]); softmax via one ACT
    Exp with accum_out row sums; P normalized then transposed on the PE
    (3x 128x128) so the AV matmuls can contract over key tokens,
    producing A^T = attention-output-transposed, which feeds the output
    projection directly as the stationary operand.
"""

import numpy as np
import ml_dtypes

BF16 = ml_dtypes.bfloat16

DIM, H, DK, DV, BS = 1536, 8, 64, 192, 128
B, N = 2, 4096
NB = 10            # blocks per core including 1-block halo each side
NT = NB * BS       # 1280 tokens of x per core
NQ = 8             # central query blocks per core
ND = DIM // 128    # 12 chunks of the contraction dim
NCORES = 8

_CACHE = {}


def _build_bass():
    import concourse.bass as bass
    import concourse.mybir as mybir
    import concourse.tile as tile
    from concourse.vector_clock import ScopedClock
    from concourse.masks import make_identity

    dt = mybir.dt
    AF = mybir.ActivationFunctionType

    class PatchedTileContext(tile.TileContext):
        # The walrus in this image rejects instructions carrying more than
        # one sync wait; hoist extras onto single-wait NOPs placed just
        # before the instruction on the same engine (engine executes in
        # program order, so wait-then-wait == wait-on-both).
        def _add_instruction(self, inst):
            si = inst.sync_info
            if (
                si is not None
                and si.on_wait
                and len(si.on_wait) > 1
                and inst.engine != mybir.EngineType.Unassigned
            ):
                waits = list(si.on_wait)
                si.on_wait = [waits[-1]]
                for w in waits[:-1]:
                    nop = mybir.InstNoOp(
                        name=f"I-{self.nc.next_id()}",
                        ins=[],
                        outs=[],
                        bass_nofuse=True,
                    )
                    nop.engine = inst.engine
                    nop.sync_info = mybir.SyncInfo(on_wait=[w], on_update=[])
                    super()._add_instruction(nop)
            super()._add_instruction(inst)

        def _drain_and_barrier(self, tick_clock, wait_clock):
            carrier = self.nc.sync.nop(nofuse=True)
            wait_clock.add_sem_waits(
                carrier.ins, ScopedClock({None: tick_clock.global_clock})
            )
            si = carrier.ins.sync_info
            waits = list(si.on_wait) if si and si.on_wait else []
            if len(waits) > 1:
                si.on_wait = [waits[0]]
                for w in waits[1:]:
                    extra = self.nc.sync.nop(nofuse=True)
                    esi = extra.ins.sync_info
                    if esi is None:
                        extra.ins.sync_info = mybir.SyncInfo(
                            on_wait=[w], on_update=[]
                        )
                    else:
                        esi.on_wait = [w]
            self.nc.sync.drain()
            self.nc.all_engine_barrier()
            popped = self.nc._tile_sem_poison_stack.pop()
            assert popped is self._sem_poison
            self.nc.clear_and_free_semaphores(
                list(self.sems.allocated().values())
            )
            self.nc.all_engine_barrier()

    nc = bass.Bass(
        "TRN2", target_bir_lowering=False, debug=False, num_devices=NCORES
    )
    xT_d = nc.dram_tensor("xT", [DIM, NT], dt.bfloat16, kind="ExternalInput")
    Wq_d = nc.dram_tensor("Wq", [DIM, H * DK], dt.bfloat16, kind="ExternalInput")
    Wk_d = nc.dram_tensor("Wk", [DIM, H * DK], dt.bfloat16, kind="ExternalInput")
    Wv_d = nc.dram_tensor("Wv", [DIM, H * DV], dt.bfloat16, kind="ExternalInput")
    Wo_d = nc.dram_tensor("Wo", [H * DV, DIM], dt.bfloat16, kind="ExternalInput")
    bo_d = nc.dram_tensor("bo_b", [128, DIM], dt.float32, kind="ExternalInput")
    y_d = nc.dram_tensor("y", [NQ * BS, DIM], dt.float32, kind="ExternalOutput")

    with PatchedTileContext(nc) as tc:
        with (
            tc.tile_pool(name="res", bufs=1) as res,
            tc.tile_pool(name="vpool", bufs=5) as vpool,
            tc.tile_pool(name="work", bufs=3) as work,
            tc.tile_pool(name="small", bufs=4) as small,
            tc.tile_pool(name="outp", bufs=2) as outp,
            tc.tile_pool(name="pp", bufs=2, space="PSUM") as pp,
            tc.tile_pool(name="ps", bufs=2, space="PSUM") as ps,
            tc.tile_pool(name="ppt", bufs=2, space="PSUM") as ppt,
            tc.tile_pool(name="pat", bufs=1, space="PSUM") as pat,
        ):
            # ---- resident tiles + input DMA ----
            xT_sb = res.tile([128, ND, NT], dt.bfloat16, tag="xT")
            Wq_sb = res.tile([128, ND, H * DK], dt.bfloat16, tag="Wq")
            Wk_sb = res.tile([128, ND, H * DK], dt.bfloat16, tag="Wk")
            Wv_sb = res.tile([128, ND, H * DV], dt.bfloat16, tag="Wv")
            Wo_sb = res.tile([128, ND, DIM], dt.bfloat16, tag="Wo")
            bo_sb = res.tile([128, DIM], dt.float32, tag="bo")
            kT_sb = res.tile([128, H // 2, NT], dt.bfloat16, tag="kT")
            qT_sb = res.tile([128, H // 2, NQ * BS], dt.bfloat16, tag="qT")
            ident = res.tile([128, 128], dt.bfloat16, tag="ident")

            make_identity(nc, ident[:])
            # DMA order is the startup critical path: the k/q projections
            # need xT (first token group) + Wk/Wq first.  All transfers go
            # through the SWDGE queue (the HWDGE queue on this image runs
            # at ~2.7 GB/s per engine); spread the enqueue work across
            # three engines so the critical loads are in flight early.
            for d in range(ND):
                nc.scalar.dma_start(
                    xT_sb[:, d, 0:512], xT_d[d * 128:(d + 1) * 128, 0:512]
                )
            for d in range(ND):
                nc.gpsimd.dma_start(Wk_sb[:, d, :], Wk_d[d * 128:(d + 1) * 128, :])
            for d in range(ND):
                nc.gpsimd.dma_start(Wq_sb[:, d, :], Wq_d[d * 128:(d + 1) * 128, :])
            for tok0, w in [(512, 512), (1024, 256)]:
                for d in range(ND):
                    nc.gpsimd.dma_start(
                        xT_sb[:, d, tok0:tok0 + w],
                        xT_d[d * 128:(d + 1) * 128, tok0:tok0 + w],
                    )
            for d in range(ND):
                nc.gpsimd.dma_start(Wv_sb[:, d, :], Wv_d[d * 128:(d + 1) * 128, :])
            nc.gpsimd.dma_start(bo_sb[:], bo_d[:])
            for d in range(ND):
                nc.gpsimd.dma_start(Wo_sb[:, d, :], Wo_d[d * 128:(d + 1) * 128, :])

            # ---- transposed projections (kT over all 10 blocks, qT over
            #      the 8 central blocks); head pair hp holds heads
            #      (2hp, 2hp+1) at partitions (0:64, 64:128) ----
            def proj_T(dst_sb, W_sb, groups, toff0):
                for tok0, w in groups:
                    for hp in range(H // 2):
                        acc = pp.tile([128, 512], dt.float32, tag="pp")
                        for d in range(ND):
                            nc.tensor.matmul(
                                acc[:, :w],
                                W_sb[:, d, hp * 128:(hp + 1) * 128],
                                xT_sb[:, d, tok0:tok0 + w],
                                start=(d == 0),
                                stop=(d == ND - 1),
                            )
                        nc.vector.tensor_copy(
                            dst_sb[:, hp, tok0 - toff0:tok0 - toff0 + w],
                            acc[:, :w],
                        )

            proj_T(kT_sb, Wk_sb, [(0, 512), (512, 512), (1024, 256)], 0)
            proj_T(qT_sb, Wq_sb, [(128, 512), (640, 512)], 128)

            # ---- v projection for one block (natural [tok, h*dv]) ----
            def v_proj(blk):
                vt = vpool.tile([128, H * DV], dt.bfloat16, tag="v")
                for vc in range(3):
                    acc = pp.tile([128, 512], dt.float32, tag="pp")
                    for d in range(ND):
                        nc.tensor.matmul(
                            acc[:],
                            xT_sb[:, d, blk * 128:(blk + 1) * 128],
                            Wv_sb[:, d, vc * 512:(vc + 1) * 512],
                            start=(d == 0),
                            stop=(d == ND - 1),
                        )
                    nc.vector.tensor_copy(vt[:, vc * 512:(vc + 1) * 512], acc[:])
                return vt

            # ---- attention + output projection for query blocks
            #      (qb, qb+1); AV matmuls are batched over the two query
            #      blocks (the shared k-blocks contribute to both via a
            #      strided rhs AP, N=256) ----
            def attention_pair(qb, vts):
                at_sbs = {
                    q2: work.tile(
                        [128, H * DV], dt.bfloat16, tag="at", name="at_sb"
                    )
                    for q2 in (qb, qb + 1)
                }
                for g in range(H // 2):
                    h0c, h1c = 2 * g * DV, (2 * g + 1) * DV
                    pts = []
                    for sub in range(2):
                        p0 = 64 * sub
                        # pt_ps columns: (qb: c0 c1 c2)(qb+1: c0 c1 c2)
                        pt_ps = ppt.tile([128, 768], dt.bfloat16, tag="ppt")
                        for qbi in range(2):
                            q2 = qb + qbi
                            s_ps = ps.tile([128, 384], dt.float32, tag="ps")
                            nc.tensor.matmul(
                                s_ps[:],
                                qT_sb[p0:p0 + 64, g, (q2 - 1) * 128:q2 * 128],
                                kT_sb[p0:p0 + 64, g,
                                      (q2 - 1) * 128:(q2 + 2) * 128],
                                start=True,
                                stop=True,
                            )
                            p_sb = work.tile([128, 384], dt.float32, tag="p")
                            ssum = small.tile([128, 1], dt.float32, tag="ssum")
                            nc.scalar.activation(
                                p_sb[:], s_ps[:], AF.Exp,
                                bias=0.0, scale=0.125, accum_out=ssum[:],
                            )
                            rinv = small.tile([128, 1], dt.float32, tag="rinv")
                            nc.vector.reciprocal(rinv[:], ssum[:])
                            pn_sb = work.tile([128, 384], dt.bfloat16, tag="pn")
                            nc.vector.tensor_scalar_mul(
                                pn_sb[:], p_sb[:], rinv[:]
                            )
                            for c in range(3):
                                nc.tensor.transpose(
                                    pt_ps[:, qbi * 384 + c * 128:
                                          qbi * 384 + (c + 1) * 128],
                                    pn_sb[:, c * 128:(c + 1) * 128],
                                    ident[:],
                                )
                        pt_sb = work.tile([128, 768], dt.bfloat16, tag="pt")
                        nc.scalar.copy(pt_sb[:], pt_ps[:])
                        pts.append(pt_sb)

                    # A^T psum for this pair: chunk r at cols r*256, holding
                    # (qb | qb+1) side by side; chunk 0 = h0 dv0:128,
                    # chunk 1 = h0 dv128:192 (parts 0:64) + h1 dv0:64
                    # (parts 64:128), chunk 2 = h1 dv64:192.
                    at_ps = pat.tile([128, 768], dt.float32, tag="pat")
                    chains = [
                        (0, h0c, 128, 0, 0, 128, None),
                        (0, h0c + 128, 64, 256, 0, 64, None),
                        (1, h1c, 64, 256, 64, 128, (0, 64)),
                        (1, h1c + 64, 128, 512, 0, 128, None),
                    ]
                    for sub, vc0, wdt, oc0, plo, phi, tpos in chains:
                        pt_sb = pts[sub]
                        pt_re = pt_sb[:].rearrange(
                            "p (a b c) -> p b a c", a=3, b=2, c=128
                        )
                        for j in range(4):
                            vt = vts[qb - 1 + j]
                            if j == 0:
                                rhs = pt_sb[:, 0:128]
                                out_ap = at_ps[plo:phi, oc0:oc0 + 128]
                            elif j == 1:
                                rhs = pt_re[:, 1, 0:2, :]
                                out_ap = at_ps[plo:phi, oc0:oc0 + 256]
                            elif j == 2:
                                rhs = pt_re[:, 0, 1:3, :]
                                out_ap = at_ps[plo:phi, oc0:oc0 + 256]
                            else:
                                rhs = pt_sb[:, 640:768]
                                out_ap = at_ps[plo:phi, oc0 + 128:oc0 + 256]
                            nc.tensor.matmul(
                                out_ap,
                                vt[:, vc0:vc0 + wdt],
                                rhs,
                                start=(j == 0),
                                stop=(j == 3),
                                tile_position=tpos,
                            )
                    at_re = at_ps[:].rearrange(
                        "p (r q x) -> p q r x", r=3, q=2, x=128
                    )
                    for qbi in range(2):
                        nc.vector.tensor_copy(
                            at_sbs[qb + qbi][:, g * 384:(g + 1) * 384],
                            at_re[:, qbi],
                        )

                for q2 in (qb, qb + 1):
                    at_sb = at_sbs[q2]
                    o_sb = outp.tile([128, DIM], dt.float32, tag="o")
                    for n in range(3):
                        acc = pp.tile([128, 512], dt.float32, tag="pp")
                        for j in range(ND):
                            nc.tensor.matmul(
                                acc[:],
                                at_sb[:, j * 128:(j + 1) * 128],
                                Wo_sb[:, j, n * 512:(n + 1) * 512],
                                start=(j == 0),
                                stop=(j == ND - 1),
                            )
                        nc.vector.tensor_add(
                            o_sb[:, n * 512:(n + 1) * 512],
                            acc[:],
                            bo_sb[:, n * 512:(n + 1) * 512],
                        )
                    nc.gpsimd.dma_start(y_d[(q2 - 1) * 128:q2 * 128, :], o_sb[:])

            vts = {}
            for blk in range(3):
                vts[blk] = v_proj(blk)
            vts[3] = v_proj(3)
            attention_pair(1, vts)
            for bp in range(1, 4):
                qb = 1 + 2 * bp
                vts[qb + 1] = v_proj(qb + 1)
                vts[qb + 2] = v_proj(qb + 2)
                attention_pair(qb, vts)

    return nc


def _get_nc():
    if "nc" not in _CACHE:
        _CACHE["nc"] = _build_bass()
    return _CACHE["nc"]


def _shard_inputs(x, Wq, Wk, Wv, Wo, bo):
    xp = np.zeros((B, N + 2 * BS, DIM), np.float32)
    xp[:, BS:BS + N] = np.asarray(x, np.float32)
    Wq_b = np.asarray(Wq).astype(BF16)
    Wk_b = np.asarray(Wk).astype(BF16)
    Wv_b = np.asarray(Wv).astype(BF16)
    Wo_b = np.asarray(Wo).astype(BF16)
    bo_b = np.ascontiguousarray(
        np.broadcast_to(np.asarray(bo, np.float32), (128, DIM))
    )
    in_maps = []
    for c in range(NCORES):
        b, ch = divmod(c, 4)
        xs = xp[b, ch * NQ * BS: ch * NQ * BS + NT]  # [1280, 1536]
        xT = np.ascontiguousarray(xs.T).astype(BF16)
        in_maps.append({
            "xT": xT, "Wq": Wq_b, "Wk": Wk_b, "Wv": Wv_b, "Wo": Wo_b,
            "bo_b": bo_b,
        })
    return in_maps


def kernel(x, Wq, Wk, Wv, Wo, bo):
    from concourse.bass_utils import run_bass_kernel_spmd

    nc = _get_nc()
    in_maps = _shard_inputs(x, Wq, Wk, Wv, Wo, bo)
    res = run_bass_kernel_spmd(nc, in_maps, list(range(NCORES)))
    out = np.empty((B, N, DIM), np.float32)
    for c in range(NCORES):
        b, ch = divmod(c, 4)
        out[b, ch * NQ * BS:(ch + 1) * NQ * BS] = res.results[c]["y"]
    return out
